# revision 2
# baseline (speedup 1.0000x reference)
"""Trainium2 Bass kernel for nn_DiffusionActionHead (MoE-style category routing).

Strategy (host side, inside kernel()):
  - Group the B=32 batch items by cat_id into token groups of <= IPS items.
    Each group's work is split into two column-halves (output-dim split of the
    big matmuls), giving uniform "half-unit" slots. Slots are distributed
    round-robin over the 8 NeuronCores; every core runs the SAME program over
    NSLOT slots (SPMD). Dummy padding slots replicate slot 0; outputs discarded.
  - All weight tables are cast to bf16 (optionally E3M4 fp8 for the
    lower-sensitivity tables, with power-of-2 scales folded into neighbouring
    tensors host-side so no extra device work is needed). This halves/quarters
    HBM traffic vs fp32 — the kernel is DMA-bandwidth-bound (~360 GB/s/core).
  - Per-item sinusoidal timestep embeddings are computed on host (function of
    the int timesteps input only); all weight-table FLOPs run on device.
  - Column-half partial outputs are summed on host during unsharding.

Device program per slot (raw Bass, manual semaphores):
  SE1  hT = relu(seW1h^T @ state + b1h)        (4x [128,IPS] matmuls)
  SE2  sf = hT^T @ seW2h (+ sb2)               (partial state_feat, 3 o-tiles)
  AE1  aT = (W1 chunks)^T @ actionsT + b1      (12x [128,TOK], transposed out)
  TT   tt = tauT^T @ W2bh (+ b2)               (per-item tau contribution)
  X2   x2 = aT^T @ W2ah + broadcast(tt); swish (2 o-tiles of 384)
  TR   x2T chunks via PE transpose
  AE3  out = x2T^T @ W3h (+ b3)                (partial, 3 o-tiles of 512)

Weight chunks stream through a ring of SBUF buffers; input DMAs ride the SP
HWDGE queue (per-buffer dma sems), secondary prefetches + output DMAs ride the
ACT HWDGE queue so both queues keep the 16 shared DMA engines saturated.
"""
import sys

sys.path.insert(0, "/opt/trn_rl_repo")

import contextlib
import numpy as np
import ml_dtypes

import concourse.bass as bass
import concourse.mybir as mybir
from concourse.bass_utils import run_bass_kernel_spmd

F32 = mybir.dt.float32
F32R = mybir.dt.float32r
BF16 = mybir.dt.bfloat16
FP8 = mybir.dt.float8e3
AF = mybir.ActivationFunctionType

NP_BF16 = ml_dtypes.bfloat16
NP_FP8 = ml_dtypes.float8_e3m4

E, STATE_DIM, ACT_DIM, HID, EMB = 32, 64, 32, 1024, 1536
B, T = 32, 32
N_CORES = 8
HH = HID // 2               # 512: h-column half for the state encoder
OH = EMB // 2               # 768: output-column half for the action encoder
RS = 6                      # SP-queue ring slots of [128, 4608] bf16
RA = 6                      # ACT-queue ring slots

# ---- dtype config -----------------------------------------------------------
# E3M4 fp8 (4 mantissa bits, ~1.3% RMS) for the low-sensitivity tables; bf16
# (~0.17% RMS) elsewhere. Scales are exact powers of two folded into
# neighbouring tensors on the host, so the device needs no descale ops except
# the sigmoid's ACT scale.
FP8_WSE2 = False
FP8_W2A = False
FP8_W2B = False
MIXED = True        # bf16 stationary operand against fp8 moving operand
S_W = 128           # fp8 quantization scale (sigma 0.02 -> 2.56)

S_AT = 32 if (FP8_W2A and not MIXED) else 1      # aT fp8 needs its own scale
S_H = 16 if (FP8_WSE2 and not MIXED) else 1      # h fp8 scale
S2 = (S_W if FP8_W2A else 1) * S_AT              # scale carried by X2 psum
S1 = (S_W if FP8_WSE2 else 1) * S_H              # scale carried by SE2 psum
TAU_SCALE = S2 / (S_W if FP8_W2B else 1)         # host scales tau by this
assert not (FP8_W2B and not MIXED), "fp8 W2b needs bf16 tau (mixed matmul)"

W2A_DT = FP8 if FP8_W2A else BF16
W2B_DT = FP8 if FP8_W2B else BF16
WSE2_DT = FP8 if FP8_WSE2 else BF16
AT_DT = FP8 if (FP8_W2A and not MIXED) else BF16
HT_DT = FP8 if (FP8_WSE2 and not MIXED) else BF16


def _sinusoid(ts):
    half = EMB // 2
    div = np.exp(-np.log(np.float32(10000.0)) * np.arange(half, dtype=np.float32) / np.float32(half))
    ang = ts.astype(np.float32)[:, None] * div[None, :]
    return np.concatenate([np.sin(ang), np.cos(ang)], axis=1).astype(np.float32)


def _bf16(a):
    return np.asarray(a, np.float32).astype(NP_BF16)


def _fp8(a, scale):
    return np.clip(np.asarray(a, np.float32) * scale, -15.0, 15.0).astype(NP_FP8)


# ---------------------------------------------------------------------------
# Build-time plan. Ops live in engine streams: "dma" (SP: input DMAs),
# "pe" (matmuls/transposes), "actq" (ACT: activations AND ACT-queue DMAs),
# "dve". Sem protocol: every DMA incs its per-buffer sem by 16; every PE op
# incs s_pe by 1; every activation incs s_act by 1; every DVE op incs s_dve
# by 1. Cross-engine deps become standalone wait_ge ops.
# ---------------------------------------------------------------------------
class _Buf:
    __slots__ = ("writer", "readers")

    def __init__(self):
        self.writer = None      # (sem, value, stream)
        self.readers = []


class _Plan:
    def __init__(self):
        self.dma = []
        self.pe = []
        self.actq = []
        self.dve = []
        self.counts = {}

    def emit(self, stream, sem, mult, op, in_bufs, out_buf, force_wait=False):
        self.counts[sem] = self.counts.get(sem, 0) + 1
        tag = (sem, self.counts[sem] * mult, stream)
        deps = []
        for b in in_bufs:
            if b.writer is not None:
                deps.append(b.writer)
        if out_buf is not None:
            deps.extend(out_buf.readers)
            if out_buf.writer is not None:
                deps.append(out_buf.writer)
        m = {}
        for dsem, dval, dstream in deps:
            if dstream == stream and not force_wait:
                continue  # same engine stream: program order
            m[dsem] = max(m.get(dsem, 0), dval)
        op["waits"] = m
        getattr(self, stream).append(op)
        for b in in_bufs:
            b.readers.append(tag)
        if out_buf is not None:
            out_buf.writer = tag
            out_buf.readers = []


def build(nslot, reps=1, ips=2, with_bias=False, probe=None):
    TOK = ips * T
    PIN_TAU = 0
    PIN_ACT = 48
    PIN_ST = PIN_ACT + TOK
    PIN_W = PIN_ST + ips + (8 - (PIN_ST + ips) % 8) % 8

    nc = bass.Bass()
    P = nc.declare_dram_parameter

    # Weights are stored host-side in chunk-major SBUF layout so each phase
    # needs only 1-2 large contiguous DMAs. Big tables are split into two
    # mega-chunk groups: g0 rides the SP HWDGE queue, g1 the ACT HWDGE queue,
    # so both queues keep the shared DMA-engine pool busy.
    wsea = P("wsea", [nslot, 64, 1280], BF16, isOutput=False)      # wse1h | ae_W1
    wse2 = P("wse2", [nslot, 2, 128, 3072], WSE2_DT, isOutput=False)  # 2x2 chunks
    w2b = P("w2b", [nslot, 2, 128, 4608], W2B_DT, isOutput=False)  # 2x6 chunks
    w2a = P("w2a", [nslot, 2, 128, 4608], W2A_DT, isOutput=False)
    w3 = P("w3", [nslot, 2, 128, 4608], BF16, isOutput=False)      # 2x3 chunks
    pin = P("pin", [nslot, 128, PIN_W], BF16, isOutput=False)
    cst_i = P("cst_i", [128, 128], BF16, isOutput=False)           # identity
    cst_f = P("cst_f", [128, 192], F32R, isOutput=False)           # onesel|ones
    biasd = (P("biasd", [nslot, 128, 3872], F32R, isOutput=False)
             if with_bias else None)
    ao = P("ao", [nslot, TOK, EMB], BF16, isOutput=True)
    st = P("st", [nslot, ips, EMB], BF16, isOutput=True)

    with contextlib.ExitStack() as es:
        ec = es.enter_context
        ring = [ec(nc.sbuf_tensor(f"ring{i}", [128, 4608], BF16)) for i in range(RS)]
        ringa = [ec(nc.sbuf_tensor(f"ringa{i}", [128, 4608], BF16)) for i in range(RA)]
        pin_b = [ec(nc.sbuf_tensor(f"pin{i}", [128, PIN_W], BF16)) for i in range(2)]
        wsea_b = [ec(nc.sbuf_tensor(f"wsea{i}", [64, 1280], BF16)) for i in range(2)]
        bias_b = ([ec(nc.sbuf_tensor(f"bias{i}", [128, 3872], F32R)) for i in range(2)]
                  if with_bias else [])
        cstI = ec(nc.sbuf_tensor("cstI", [128, 128], BF16))
        cstF = ec(nc.sbuf_tensor("cstF", [128, 192], F32R))
        s_hT = ec(nc.sbuf_tensor("s_hT", [128, 4 * ips], HT_DT))
        s_aT = ec(nc.sbuf_tensor("s_aT", [128, 12 * TOK], AT_DT))
        s_tt = ec(nc.sbuf_tensor("s_tt", [ips, OH], F32R))
        s_sg = ec(nc.sbuf_tensor("s_sg", [TOK, OH], BF16))
        s_x2 = ec(nc.sbuf_tensor("s_x2", [TOK, OH], BF16))
        s_x2T = ec(nc.sbuf_tensor("s_x2T", [128, 6 * TOK], BF16))
        s_out = [ec(nc.sbuf_tensor(f"s_out{i}", [TOK, EMB], BF16)) for i in range(2)]
        s_st = [ec(nc.sbuf_tensor(f"s_st{i}", [ips, EMB], BF16)) for i in range(2)]
        pA = ec(nc.psum_tensor("pA", [128, 512], F32))
        pB0 = ec(nc.psum_tensor("pB0", [128, 512], F32))
        pB1 = ec(nc.psum_tensor("pB1", [128, 512], F32))
        pC = ec(nc.psum_tensor("pC", [128, 512], F32))
        pD = ec(nc.psum_tensor("pD", [128, 512], F32))
        pE = ec(nc.psum_tensor("pE", [128, 512], F32))
        pT = ec(nc.psum_tensor("pT", [128, 1024], BF16))
        s_pe = ec(nc.semaphore("s_pe"))
        s_act = ec(nc.semaphore("s_act"))
        s_dve = ec(nc.semaphore("s_dve"))
        block = ec(nc.Block())

        # fp8 views of the rings (free size doubles under the bitcast)
        ring8 = [r.bitcast(FP8) for r in ring]
        ringa8 = [r.bitcast(FP8) for r in ringa]

        # ---------------- plan ----------------
        pl = _Plan()
        bufs = {
            "ring": [_Buf() for _ in range(RS)],
            "ringa": [_Buf() for _ in range(RA)],
            "pin": [_Buf() for _ in range(2)],
            "wseab": [_Buf() for _ in range(2)],
            "bias": [_Buf() for _ in range(2)],
            "hT": [_Buf() for _ in range(4)],
            "aT": [_Buf() for _ in range(12)],
            "tt": [_Buf() for _ in range(2)],
            "x2": [_Buf() for _ in range(2)],
            "sg": [_Buf() for _ in range(2)],
            "x2T": [_Buf() for _ in range(6)],
            "out": [_Buf() for _ in range(2)],
            "stb": [_Buf() for _ in range(2)],
            # pA/pT are single PSUM banks: PE writes and ACT/DVE reads of the
            # same bank are fatal if concurrent (P10), so track whole-tensor —
            # each new PE write waits for the previous quarter's reader.
            "pA": _Buf(),
            "pB0": _Buf(),
            "pB1": _Buf(),
            "pC": _Buf(),
            "pD": _Buf(),
            "pE": _Buf(),
            "pT": _Buf(),
            "cstI": _Buf(),
            "cstF": _Buf(),
        }
        rc = [0]
        rca = [0]

        def next_ring():
            r = rc[0] % RS
            rc[0] += 1
            return r

        def next_ringa():
            r = rca[0] % RA
            rca[0] += 1
            return r

        def dma_in(dst, dst_sl, src, src_sl, buf, key, q="sp"):
            # per-buffer DMA sems: successive writes to one buffer are ordered
            # by the WAR chain, so "sem >= 16*n" fires exactly at write n's
            # completion; a shared cumulative sem would be unsound.
            if q == "sp":
                pl.emit("dma", "dma:" + key, 16,
                        {"dst": dst, "dst_sl": dst_sl, "src": src, "src_sl": src_sl,
                         "key": "dma:" + key},
                        [], buf)
            else:
                pl.emit("actq", "dmo:" + key, 16,
                        {"kind": "dmo", "dst": dst, "dst_sl": dst_sl, "src": src,
                         "src_sl": src_sl, "key": "dmo:" + key},
                        [], buf)

        def dma_out(dst, dst_sl, src, src_sl, buf, key):
            # on the ACT stream; force same-stream wait (DMA engines are async
            # w.r.t. the ACT pipeline, so wait for the producing copy's sem)
            pl.emit("actq", "dmo:" + key, 16,
                    {"kind": "dmo", "dst": dst, "dst_sl": dst_sl, "src": src,
                     "src_sl": src_sl, "key": "dmo:" + key}, [buf], None,
                    force_wait=True)

        def mm(out, out_sl, lhs, lhs_sl, rhs, rhs_sl, start, stop, in_bufs, out_buf):
            pl.emit("pe", "pe", 1,
                    {"kind": "mm", "out": out, "out_sl": out_sl, "lhs": lhs,
                     "lhs_sl": lhs_sl, "rhs": rhs, "rhs_sl": rhs_sl,
                     "start": start, "stop": stop}, in_bufs, out_buf)

        def tr(out, out_sl, in_, in_sl, in_bufs, out_buf):
            pl.emit("pe", "pe", 1,
                    {"kind": "tr", "out": out, "out_sl": out_sl, "in": in_,
                     "in_sl": in_sl}, in_bufs, out_buf)

        def act(out, out_sl, in_, in_sl, func, bias, scale, in_bufs, out_buf):
            pl.emit("actq", "act", 1,
                    {"kind": "act", "out": out, "out_sl": out_sl, "in": in_,
                     "in_sl": in_sl, "func": func, "bias": bias, "scale": scale},
                    in_bufs, out_buf)

        def dve(out, out_sl, in_, in_sl, in_bufs, out_buf):
            pl.emit("dve", "dve", 1,
                    {"out": out, "out_sl": out_sl, "in": in_, "in_sl": in_sl},
                    in_bufs, out_buf)

        dma_in("cstI", np.s_[:, :], "cst_i", np.s_[:, :], bufs["cstI"], "csti")
        dma_in("cstF", np.s_[:, :], "cst_f", np.s_[:, :], bufs["cstF"], "cstf")
        CS_ONE = 128  # cstF col of the all-ones row (bias broadcast matmuls)

        # names of the dtype-correct ring views for each table
        RV_SE2 = "ring8" if FP8_WSE2 else "ring"
        RVA_SE2 = "ringa8" if FP8_WSE2 else "ringa"
        RV_2B = "ring8" if FP8_W2B else "ring"
        RVA_2B = "ringa8" if FP8_W2B else "ringa"
        RV_2A = "ring8" if FP8_W2A else "ring"
        RVA_2A = "ringa8" if FP8_W2A else "ringa"

        def emit_slot(s, emit_prev_out):
            sb = s % 2
            pinb = bufs["pin"][sb]
            wseab = bufs["wseab"][sb]
            biab = bufs["bias"][sb]
            dma_in("pin_b", (sb, np.s_[:, :]), "pin", np.s_[s, :, :], pinb, f"pin{sb}")
            dma_in("wsea_b", (sb, np.s_[:, :]), "wsea", np.s_[s, :, :], wseab, f"wsea{sb}")
            if with_bias:
                dma_in("bias_b", (sb, np.s_[:, :]), "biasd", np.s_[s, :, :], biab, f"bias{sb}")

            # ---- ACT-queue prefetch: second half of each big weight phase
            ra_s = next_ringa()
            dma_in(RVA_SE2, (ra_s, np.s_[:, 0:3072]), "wse2", np.s_[s, 1, :, :],
                   bufs["ringa"][ra_s], f"ra{ra_s}", q="act")
            ra_b = next_ringa()
            dma_in(RVA_2B, (ra_b, np.s_[:, 0:4608]), "w2b", np.s_[s, 1, :, :],
                   bufs["ringa"][ra_b], f"ra{ra_b}", q="act")
            ra_a = next_ringa()
            dma_in(RVA_2A, (ra_a, np.s_[:, 0:4608]), "w2a", np.s_[s, 1, :, :],
                   bufs["ringa"][ra_a], f"ra{ra_a}", q="act")
            ra_3 = next_ringa()
            dma_in("ringa", (ra_3, np.s_[:, 0:4608]), "w3", np.s_[s, 1, :, :],
                   bufs["ringa"][ra_3], f"ra{ra_3}", q="act")

            # ---- SE1: hT[128h, ips] per k-chunk of the h-half ----
            for k in range(4):
                mm("pA", np.s_[0:128, k * ips:(k + 1) * ips],
                   "wsea_b", (sb, np.s_[0:STATE_DIM, k * 128:(k + 1) * 128]),
                   "pin_b", (sb, np.s_[0:STATE_DIM, PIN_ST:PIN_ST + ips]),
                   True, True, [wseab, pinb], bufs["pA"])
                act("s_hT", np.s_[:, k * ips:(k + 1) * ips],
                    "pA", np.s_[0:128, k * ips:(k + 1) * ips],
                    AF.Relu, ((sb, 12 + k) if with_bias else None), float(S_H),
                    [bufs["pA"]] + ([biab] if with_bias else []), bufs["hT"][k])
            # ---- SE2 (4 k-chunks: 2 on SP ring, 2 on ACT ring) ----
            r_s = next_ring()
            dma_in(RV_SE2, (r_s, np.s_[:, 0:3072]), "wse2", np.s_[s, 0, :, :],
                   bufs["ring"][r_s], f"r{r_s}")
            for k in range(4):
                gi, c = divmod(k, 2)
                rg, rn, rbuf = ((r_s, RV_SE2, bufs["ring"][r_s]) if gi == 0
                                else (ra_s, RVA_SE2, bufs["ringa"][ra_s]))
                for t, pn in enumerate(("pB0", "pB1", "pE")):
                    mm(pn, np.s_[0:ips, 0:512],
                       "s_hT", np.s_[:, k * ips:(k + 1) * ips],
                       rn, (rg, np.s_[:, c * 1536 + t * 512:c * 1536 + (t + 1) * 512]),
                       k == 0, (k == 3 and not with_bias),
                       [bufs["hT"][k], rbuf], bufs[pn])
            if with_bias:
                for t, pn in enumerate(("pB0", "pB1", "pE")):
                    mm(pn, np.s_[0:ips, 0:512],
                       "cstF", np.s_[0:1, CS_ONE:CS_ONE + ips],
                       "bias_b", (sb, np.s_[0:1, 2336 + t * 512:2336 + (t + 1) * 512]),
                       False, True, [bufs["cstF"], biab], bufs[pn])
            for t, pn in enumerate(("pB0", "pB1", "pE")):
                dve("s_st", (sb, np.s_[0:ips, t * 512:(t + 1) * 512]),
                    pn, np.s_[0:ips, 0:512], [bufs[pn]], bufs["stb"][sb])
            # ---- AE1 (weights resident in wsea_b cols 512:1280) ----
            for j in range(12):
                q = j % 8
                r0, c0 = 32 * (j // 6), 512 + (j % 6) * 128
                mm("pA", np.s_[:, q * TOK // 2:q * TOK // 2 + TOK],
                   "wsea_b", (sb, np.s_[r0:r0 + ACT_DIM, c0:c0 + 128]),
                   "pin_b", (sb, np.s_[0:ACT_DIM, PIN_ACT:PIN_ACT + TOK]),
                   True, True, [wseab, pinb], bufs["pA"])
                act("s_aT", np.s_[:, j * TOK:(j + 1) * TOK],
                    "pA", np.s_[:, q * TOK // 2:q * TOK // 2 + TOK],
                    AF.Identity, ((sb, j) if with_bias else None), float(S_AT),
                    [bufs["pA"]] + ([biab] if with_bias else []), bufs["aT"][j])
            # ---- TT (2 mega-chunks of 6 k-chunks each) ----
            r_b = next_ring()
            dma_in(RV_2B, (r_b, np.s_[:, 0:4608]), "w2b", np.s_[s, 0, :, :],
                   bufs["ring"][r_b], f"r{r_b}")
            for gi in range(2):
                rg, rn, rbuf = ((r_b, RV_2B, bufs["ring"][r_b]) if gi == 0
                                else (ra_b, RVA_2B, bufs["ringa"][ra_b]))
                for c in range(6):
                    k = gi * 6 + c
                    for t, pn in enumerate(("pB0", "pB1")):
                        mm(pn, np.s_[0:ips, 0:384],
                           "pin_b", (sb, np.s_[0:128, PIN_TAU + k * ips:PIN_TAU + (k + 1) * ips]),
                           rn, (rg, np.s_[:, c * 768 + t * 384:c * 768 + (t + 1) * 384]),
                           k == 0, (k == 11 and not with_bias),
                           [pinb, rbuf], bufs[pn])
            if with_bias:
                for t, pn in enumerate(("pB0", "pB1")):
                    mm(pn, np.s_[0:ips, 0:384],
                       "cstF", np.s_[0:1, CS_ONE:CS_ONE + ips],
                       "bias_b", (sb, np.s_[0:1, 16 + t * 384:16 + (t + 1) * 384]),
                       False, True, [bufs["cstF"], biab], bufs[pn])
            for t, pn in enumerate(("pB0", "pB1")):
                act("s_tt", np.s_[0:ips, t * 384:(t + 1) * 384],
                    pn, np.s_[0:ips, 0:384], AF.Copy, None, 1.0,
                    [bufs[pn]], bufs["tt"][t])

            # previous slot's output DMAs, ~2/3 of a slot into this slot
            emit_prev_out()

            # ---- X2 (2 mega-chunks; accumulate a@W2a then broadcast tt) ----
            r_a = next_ring()
            dma_in(RV_2A, (r_a, np.s_[:, 0:4608]), "w2a", np.s_[s, 0, :, :],
                   bufs["ring"][r_a], f"r{r_a}")
            for gi in range(2):
                rg, rn, rbuf = ((r_a, RV_2A, bufs["ring"][r_a]) if gi == 0
                                else (ra_a, RVA_2A, bufs["ringa"][ra_a]))
                for c in range(6):
                    k = gi * 6 + c
                    for t, pn in enumerate(("pC", "pD")):
                        mm(pn, np.s_[0:TOK, 0:384], "s_aT", np.s_[:, k * TOK:(k + 1) * TOK],
                           rn, (rg, np.s_[:, c * 768 + t * 384:c * 768 + (t + 1) * 384]),
                           k == 0, False, [bufs["aT"][k], rbuf], bufs[pn])
            for t, pn in enumerate(("pC", "pD")):
                mm(pn, np.s_[0:TOK, 0:384],
                   "cstF", np.s_[0:ips, 0:TOK],
                   "s_tt", np.s_[0:ips, t * 384:(t + 1) * 384],
                   False, True, [bufs["cstF"], bufs["tt"][t]], bufs[pn])
            for t, pn in enumerate(("pC", "pD")):
                # swish = x * sigmoid(x): ACT computes sigmoid, DVE multiplies.
                # psum carries S2*x; sigmoid descales, the product keeps S2
                # (folded into W3 host-side).
                act("s_sg", np.s_[:, t * 384:(t + 1) * 384], pn, np.s_[0:TOK, 0:384],
                    AF.Sigmoid, None, 1.0 / S2, [bufs[pn]], bufs["sg"][t])
                pl.emit("dve", "dve", 1,
                        {"kind": "mul",
                         "out": "s_x2", "out_sl": np.s_[:, t * 384:(t + 1) * 384],
                         "in": pn, "in_sl": np.s_[0:TOK, 0:384],
                         "in2": "s_sg", "in2_sl": np.s_[:, t * 384:(t + 1) * 384]},
                        [bufs[pn], bufs["sg"][t]], bufs["x2"][t])
            # ---- TR ----
            for t in range(6):
                q = t % 4
                tr("pT", np.s_[:, q * TOK:(q + 1) * TOK],
                   "s_x2", np.s_[0:TOK, t * 128:(t + 1) * 128],
                   [bufs["x2"][t // 3]], bufs["pT"])
                dve("s_x2T", np.s_[:, t * TOK:(t + 1) * TOK],
                    "pT", np.s_[:, q * TOK:(q + 1) * TOK],
                    [bufs["pT"]], bufs["x2T"][t])
            # ---- AE3 (2 mega-chunks of 3 k-chunks) ----
            r_3 = next_ring()
            dma_in("ring", (r_3, np.s_[:, 0:4608]), "w3", np.s_[s, 0, :, :],
                   bufs["ring"][r_3], f"r{r_3}")
            for gi in range(2):
                rg, rn, rbuf = ((r_3, "ring", bufs["ring"][r_3]) if gi == 0
                                else (ra_3, "ringa", bufs["ringa"][ra_3]))
                for c in range(3):
                    k = gi * 3 + c
                    for t, pn in enumerate(("pC", "pD", "pE")):
                        mm(pn, np.s_[0:TOK, 0:512], "s_x2T", np.s_[:, k * TOK:(k + 1) * TOK],
                           rn, (rg, np.s_[:, c * 1536 + t * 512:c * 1536 + (t + 1) * 512]),
                           k == 0, (k == 5 and not with_bias),
                           [bufs["x2T"][k], rbuf], bufs[pn])
            if with_bias:
                for t, pn in enumerate(("pC", "pD", "pE")):
                    mm(pn, np.s_[0:TOK, 0:512],
                       "cstF", np.s_[0:1, CS_ONE:CS_ONE + TOK],
                       "bias_b", (sb, np.s_[0:1, 800 + t * 512:800 + (t + 1) * 512]),
                       False, True, [bufs["cstF"], biab], bufs[pn])
            for t, pn in enumerate(("pC", "pD", "pE")):
                dve("s_out", (sb, np.s_[0:TOK, t * 512:(t + 1) * 512]),
                    pn, np.s_[0:TOK, 0:512], [bufs[pn]], bufs["out"][sb])

        def make_out_emitter(s):
            def f():
                sb = s % 2
                dma_out("ao", np.s_[s, :, :], "s_out", (sb, np.s_[:, :]),
                        bufs["out"][sb], f"out{sb}")
                dma_out("st", np.s_[s, :, :], "s_st", (sb, np.s_[:, :]),
                        bufs["stb"][sb], f"st{sb}")
            return f

        pending = lambda: None  # noqa: E731
        for rep in range(reps):
            for s in range(nslot):
                emit_slot(s, pending)
                pending = make_out_emitter(s)
        pending()

        # ---------------- emit ----------------
        dma_sems = {k: ec(nc.semaphore("sem_" + k.replace(":", "_")))
                    for k in pl.counts if k.startswith(("dma:", "dmo:"))}

        tensors = {
            "ring": ring, "ringa": ringa, "ring8": ring8, "ringa8": ringa8,
            "pin_b": pin_b, "wsea_b": wsea_b, "bias_b": bias_b,
            "cstI": cstI, "cstF": cstF,
            "s_hT": s_hT, "s_aT": s_aT,
            "s_tt": s_tt, "s_x2": s_x2, "s_sg": s_sg, "s_x2T": s_x2T,
            "s_out": s_out, "s_st": s_st,
            "pA": pA, "pB0": pB0, "pB1": pB1, "pC": pC, "pD": pD, "pE": pE, "pT": pT,
            "wsea": wsea, "wse2": wse2, "w2b": w2b, "w2a": w2a,
            "w3": w3, "pin": pin, "biasd": biasd, "cst_i": cst_i, "cst_f": cst_f,
            "ao": ao, "st": st,
        }

        def ap(name, sl):
            t = tensors[name]
            if isinstance(t, list):
                i, s2_ = sl
                return t[i][s2_]
            return t[sl]

        sems = {"pe": s_pe, "act": s_act, "dve": s_dve}

        def make_waiter(eng_handle):
            hw = {}

            def wait(wmap):
                for sname in sorted(wmap):
                    val = wmap[sname]
                    if hw.get(sname, 0) >= val:
                        continue
                    hw[sname] = val
                    h = sems[sname] if sname in sems else dma_sems[sname]
                    eng_handle.wait_ge(h, val)

            return wait

        if probe == "pe":
            pl.dma = []
        if probe in ("dma", "pe"):
            for _lst in (pl.dma, pl.pe, pl.actq, pl.dve):
                for _op in _lst:
                    _op["waits"] = {}
        if probe == "dma":
            # self-throttle: each DMA waits for the previous write to its own
            # buffer (ring depth flow control without compute)
            _kc = {}
            for _op in pl.dma:
                _k = _op["key"]
                if _kc.get(_k, 0) > 0:
                    _op["waits"] = {_k: 16 * _kc[_k]}
                _kc[_k] = _kc.get(_k, 0) + 1
        if probe == "dma":
            pl.pe = []
            pl.actq = [o for o in pl.actq if o["kind"] != "act"]
            pl.dve = [{"out": "s_hT", "out_sl": np.s_[0:4, 0:4],
                       "in": op["dst"],
                       "in_sl": op["dst_sl"],
                       "probe_read": True, "waits": {}}
                      for op in pl.dma]
            for op in pl.dve:
                sl = op["in_sl"]
                if isinstance(sl, tuple) and isinstance(sl[0], int):
                    op["in_sl"] = (sl[0], np.s_[0:4, 0:4])
                else:
                    op["in_sl"] = np.s_[0:4, 0:4]
        if probe == "pe":
            pl.actq = []
            pl.dve = []

        @block.sync
        def _(sync):
            wait = make_waiter(sync)
            cnt = {}
            for op in pl.dma:
                wait(op["waits"])
                k = op["key"]
                cnt[k] = cnt.get(k, 0) + 16
                sync.dma_start(out=ap(op["dst"], op["dst_sl"]),
                               in_=ap(op["src"], op["src_sl"])).then_inc(dma_sems[k], 16)
            for k, v in sorted(cnt.items()):
                sync.wait_ge(dma_sems[k], v)

        @block.tensor
        def _(pe):
            wait = make_waiter(pe)
            for op in pl.pe:
                wait(op["waits"])
                if op["kind"] == "mm":
                    pe.matmul(ap(op["out"], op["out_sl"]), ap(op["lhs"], op["lhs_sl"]),
                              ap(op["rhs"], op["rhs_sl"]), start=op["start"],
                              stop=op["stop"]).then_inc(s_pe, 1)
                else:
                    pe.transpose(ap(op["out"], op["out_sl"]), ap(op["in"], op["in_sl"]),
                                 cstI[0:TOK, 0:TOK]).then_inc(s_pe, 1)

        @block.scalar
        def _(a):
            wait = make_waiter(a)
            dmo_cnt = {}
            for op in pl.actq:
                wait(op["waits"])
                if op["kind"] == "dmo":
                    k = op["key"]
                    dmo_cnt[k] = dmo_cnt.get(k, 0) + 16
                    a.dma_start(out=ap(op["dst"], op["dst_sl"]),
                                in_=ap(op["src"], op["src_sl"])).then_inc(dma_sems[k], 16)
                elif op["bias"] is None:
                    a.activation(ap(op["out"], op["out_sl"]), ap(op["in"], op["in_sl"]),
                                 op["func"], scale=op["scale"]).then_inc(s_act, 1)
                else:
                    bi, bc = op["bias"]
                    bias_ap = bias_b[bi][:, bc:bc + 1].bitcast(F32)
                    a.activation(ap(op["out"], op["out_sl"]), ap(op["in"], op["in_sl"]),
                                 op["func"], bias=bias_ap,
                                 scale=op["scale"]).then_inc(s_act, 1)
            for k, v in sorted(dmo_cnt.items()):
                a.wait_ge(dma_sems[k], v)

        @block.vector
        def _(v):
            wait = make_waiter(v)
            for op in pl.dve:
                wait(op["waits"])
                if op.get("kind") == "mul":
                    v.tensor_mul(ap(op["out"], op["out_sl"]),
                                 ap(op["in"], op["in_sl"]),
                                 ap(op["in2"], op["in2_sl"])).then_inc(s_dve, 1)
                else:
                    v.tensor_copy(ap(op["out"], op["out_sl"]),
                                  ap(op["in"], op["in_sl"])).then_inc(s_dve, 1)

    return nc


# ---------------------------------------------------------------------------
# Host-side routing, gathering, execution, unsharding
# ---------------------------------------------------------------------------
def plan_all(cat_ids):
    """Group items by category, split into (cat, items, half) units, balance
    over cores. Returns (units, per_core, nslot, ips)."""
    order = {}
    for b, g in enumerate(cat_ids.tolist()):
        order.setdefault(g, []).append(b)
    ips = max(1, min(4, max(len(v) for v in order.values())))
    units = []
    for g in sorted(order):
        items = order[g]
        for i0 in range(0, len(items), ips):
            grp = items[i0:i0 + ips]
            for h in range(2):
                units.append((g, grp, h))
    nslot = max(1, -(-len(units) // N_CORES))
    per_core = [[] for _ in range(N_CORES)]
    for i, u in enumerate(units):
        per_core[i % N_CORES].append(u)
    for c in range(N_CORES):
        while len(per_core[c]) < nslot:
            per_core[c].append(None)  # dummy
    return units, per_core, nslot, ips


def make_inputs(units_core, nslot, ips, state, actions, tau_np,
                se_W1, se_b1, se_W2, se_b2,
                ae_W1, ae_b1, ae_W2, ae_b2, ae_W3, ae_b3, with_bias=False):
    TOK = ips * T
    PIN_TAU = 0
    PIN_ACT = 48
    PIN_ST = PIN_ACT + TOK
    PIN_W = PIN_ST + ips + (8 - (PIN_ST + ips) % 8) % 8
    z = np.zeros
    f = np.float32
    cst_i = np.eye(128, dtype=f).astype(NP_BF16)
    cst_f = z((128, 192), f)
    cst_f[0:ips, 0:TOK] = np.kron(np.eye(ips, dtype=f), np.ones((1, T), f))
    cst_f[0, 128:] = 1.0
    d = {
        "wsea": z((nslot, 64, 1280), NP_BF16),
        "wse2": z((nslot, 2, 128, 3072), NP_FP8 if FP8_WSE2 else NP_BF16),
        "w2b": z((nslot, 2, 128, 4608), NP_FP8 if FP8_W2B else NP_BF16),
        "w2a": z((nslot, 2, 128, 4608), NP_FP8 if FP8_W2A else NP_BF16),
        "w3": z((nslot, 2, 128, 4608), NP_BF16),
        "pin": z((nslot, 128, PIN_W), NP_BF16),
        "cst_i": cst_i,
        "cst_f": cst_f,
    }

    def chunk_major(w, groups, chunks, width):
        # [groups*chunks*128, width] -> [groups, 128, chunks*width]
        return (w.reshape(groups, chunks, 128, width)
                .transpose(0, 2, 1, 3).reshape(groups, 128, chunks * width))
    if with_bias:
        d["biasd"] = z((nslot, 128, 3872), f)
    for s, (g, items, h) in enumerate(units_core):
        H = slice(h * HH, (h + 1) * HH)
        O = slice(h * OH, (h + 1) * OH)
        d["wsea"][s][:STATE_DIM, 0:HH] = _bf16(se_W1[g][:, H])
        for j in range(12):
            r0, c0 = 32 * (j // 6), 512 + (j % 6) * 128
            d["wsea"][s][r0:r0 + ACT_DIM, c0:c0 + 128] = _bf16(ae_W1[g][:, j * 128:(j + 1) * 128])
        se2 = chunk_major(se_W2[g][H, :], 2, 2, EMB)
        w2bg = chunk_major(ae_W2[g][EMB:, O], 2, 6, OH)
        w2ag = chunk_major(ae_W2[g][:EMB, O], 2, 6, OH)
        d["wse2"][s] = _fp8(se2, S_W) if FP8_WSE2 else _bf16(se2)
        d["w2b"][s] = _fp8(w2bg, S_W) if FP8_W2B else _bf16(w2bg)
        d["w2a"][s] = _fp8(w2ag, S_W) if FP8_W2A else _bf16(w2ag)
        d["w3"][s] = _bf16(chunk_major(ae_W3[g][O, :], 2, 3, EMB) * (1.0 / S2))
        p = np.zeros((128, PIN_W), f)
        tau3 = p[:, PIN_TAU:PIN_TAU + 12 * ips].reshape(128, 12, ips)
        for i, b in enumerate(items):
            tau3[:, :, i] = (tau_np[b] * TAU_SCALE).reshape(12, 128).T
            p[0:ACT_DIM, PIN_ACT + i * T:PIN_ACT + (i + 1) * T] = actions[b].T
            p[0:STATE_DIM, PIN_ST + i] = state[b, 0]
        d["pin"][s] = _bf16(p)
        if with_bias:
            bb = d["biasd"][s]
            for j in range(12):
                bb[:, j] = S_AT * ae_b1[g][j * 128:(j + 1) * 128]
            for k in range(4):
                bb[:, 12 + k] = S_H * se_b1[g][H][k * 128:(k + 1) * 128]
            bb[0, 16:16 + OH] = S2 * ae_b2[g][O]
            if h == 0:
                bb[0, 800:800 + EMB] = ae_b3[g]
                bb[0, 2336:2336 + EMB] = S1 * se_b2[g]
    return d


def kernel(state, actions, timesteps, cat_ids,
           se_W1, se_b1, se_W2, se_b2,
           ae_W1, ae_b1, ae_W2, ae_b2, ae_W3, ae_b3):
    args = [np.asarray(a) for a in (state, actions, timesteps, cat_ids, se_W1, se_b1,
                                    se_W2, se_b2, ae_W1, ae_b1, ae_W2, ae_b2, ae_W3, ae_b3)]
    (state, actions, timesteps, cat_ids, se_W1, se_b1, se_W2, se_b2,
     ae_W1, ae_b1, ae_W2, ae_b2, ae_W3, ae_b3) = args
    tau_np = _sinusoid(timesteps)

    units, per_core, nslot, ips = plan_all(cat_ids)
    with_bias = bool(any(np.any(a) for a in (se_b1, ae_b1, ae_b2, ae_b3, se_b2)))
    in_maps = []
    for c in range(N_CORES):
        units_c = [(u if u is not None else units[0]) for u in per_core[c]]
        in_maps.append(make_inputs(units_c, nslot, ips, state, actions, tau_np,
                                   se_W1, se_b1, se_W2, se_b2,
                                   ae_W1, ae_b1, ae_W2, ae_b2, ae_W3, ae_b3,
                                   with_bias=with_bias))

    nc = build(nslot, ips=ips, with_bias=with_bias)
    res = run_bass_kernel_spmd(nc, in_maps, list(range(N_CORES)))

    out = np.zeros((B, T + 1, EMB), np.float32)
    st_scale = 1.0 / S1
    for c in range(N_CORES):
        ao = res.results[c]["ao"].astype(np.float32)
        stx = res.results[c]["st"].astype(np.float32)
        for s, u in enumerate(per_core[c]):
            if u is None:
                continue
            g, items, h = u
            for i, b in enumerate(items):
                out[b, 0] += stx[s, i] * st_scale
                out[b, 1:] += ao[s, i * T:(i + 1) * T]
    return out
